# revision 16
# baseline (speedup 1.0000x reference)
"""Trainium2 Bass kernel for the DNC MemoryAccess module.

Sharding: data-parallel over batch B=8 -> one NeuronCore per batch element.
Each core streams its [NW, M, M] link-matrix slice through SBUF in
[128, M] row tiles, applying the fused update
    new_link[i,j] = ((1 - w_j) - w_i) * prev_link[i,j] + w_i * prev_prec[j]
with two scalar_tensor_tensor ops per tile, then zeroes the diagonal block
with a precomputed [128,128] mask. All small outputs (cosine weights,
precedence, usage, erase/write, read) run on the tensor/activation engines
and overlap with the link DMA stream.
"""

from contextlib import ExitStack

import numpy as np

import concourse.bass as bass
import concourse.tile as tile
from concourse import bacc, mybir
from concourse.bass_utils import run_bass_kernel_spmd
from concourse.masks import make_identity

B, M, W, NW, NR, H = 8, 2048, 64, 2, 4, 4
EPS = 1e-6
P = 128
NT = M // P  # 16 row tiles per link matrix
F32 = mybir.dt.float32
AX = mybir.AxisListType
OP = mybir.AluOpType
ACT = mybir.ActivationFunctionType


def _emit(ctx: ExitStack, tc: tile.TileContext, ins: dict, outs: dict):
    nc = tc.nc

    const = ctx.enter_context(tc.tile_pool(name="const", bufs=1))
    bcast = ctx.enter_context(tc.tile_pool(name="bcast", bufs=1))
    small = ctx.enter_context(tc.tile_pool(name="small", bufs=1))
    psA = ctx.enter_context(tc.tile_pool(name="psA", bufs=3, space="PSUM"))
    psW = ctx.enter_context(tc.tile_pool(name="psW", bufs=1, space="PSUM"))
    lpool = ctx.enter_context(tc.tile_pool(name="lin", bufs=3))
    opool = ctx.enter_context(tc.tile_pool(name="lout", bufs=3))

    # ---------------- constants / small input loads ----------------
    ww = const.tile([NW, M], F32, tag="ww")
    nc.sync.dma_start(ww[:], ins["write_weights"][:, :])
    rw = const.tile([NR, M], F32, tag="rw")
    nc.sync.dma_start(rw[:], ins["read_weights"][:, :])
    pp = const.tile([NW, M], F32, tag="pp")
    nc.sync.dma_start(pp[:], ins["prev_precedence"][:, :])
    pu = const.tile([1, M], F32, tag="pu")
    nc.sync.dma_start(pu[:], ins["prev_usage"][:, :])
    fg = const.tile([NR, 1], F32, tag="fg")
    nc.sync.dma_start(fg[:], ins["free_gate"][:, :])
    ev = const.tile([NW, W], F32, tag="ev")
    nc.sync.dma_start(ev[:], ins["erase_vectors"][:, :])
    wv = const.tile([NW, W], F32, tag="wv")
    nc.sync.dma_start(wv[:], ins["write_vectors"][:, :])
    rv = const.tile([NR, W], F32, tag="rv")
    nc.sync.dma_start(rv[:], ins["read_vectors"][:, :])
    keys = const.tile([H, W], F32, tag="keys")
    nc.sync.dma_start(keys[:], ins["keys"][:, :])
    stg = const.tile([H, 1], F32, tag="stg")
    nc.sync.dma_start(stg[:], ins["strengths"][:, :])
    # keys transposed [W, H] for the content-dot matmul
    keysT = const.tile([W, H], F32, tag="keysT")
    nc.sync.dma_start(keysT[:], ins["keys"].rearrange("h w -> w h"))
    # write weights in column layout: w_col[p, n*NT + t] = ww[n, t*128 + p]
    w_col = const.tile([P, NW * NT], F32, tag="w_col")
    for n in range(NW):
        nc.sync.dma_start(
            w_col[:, n * NT : (n + 1) * NT],
            ins["write_weights"][n].rearrange("(t p) -> p t", p=P),
        )
    # memory, packed [128, NT*W] (block t at cols [t*W, (t+1)*W))
    mem = const.tile([P, NT * W], F32, tag="mem")
    nc.sync.dma_start(
        mem[:].rearrange("p (t w) -> p t w", t=NT),
        ins["memory"].rearrange("(t p) w -> p t w", p=P),
    )

    # identity for PE transpose + diagonal-zero mask
    ident = const.tile([P, P], F32, tag="ident")
    make_identity(nc, ident[:])
    dmask = const.tile([P, P], F32, tag="dmask")
    nc.gpsimd.memset(dmask[:], 1.0)
    nc.gpsimd.affine_select(
        out=dmask[:], in_=dmask[:], compare_op=OP.not_equal, fill=0.0,
        base=0, pattern=[[-1, P]], channel_multiplier=1,
    )
    ones_w_h = const.tile([W, H], F32, tag="ones_w_h")
    nc.gpsimd.memset(ones_w_h[:], 1.0)
    ones_nw = const.tile([NW, 1], F32, tag="ones_nw")
    nc.gpsimd.memset(ones_nw[:], 1.0)
    ones_nr = const.tile([NR, 1], F32, tag="ones_nr")
    nc.gpsimd.memset(ones_nr[:], 1.0)

    # ---------------- link-loop broadcast tiles ----------------
    # a_bc[n][p, j] = 1 - ww[n, j], p_bc[n][p, j] = prev_prec[n, j]
    a_bc, p_bc = [], []
    for n in range(NW):
        ab = bcast.tile([P, M], F32, tag=f"ab{n}")
        nc.sync.dma_start(
            ab[:], ins["write_weights"][n : n + 1, :].to_broadcast([P, M])
        )
        nc.scalar.activation(ab[:], ab[:], ACT.Copy, bias=1.0, scale=-1.0)
        a_bc.append(ab)
        pb = bcast.tile([P, M], F32, tag=f"pb{n}")
        nc.sync.dma_start(
            pb[:], ins["prev_precedence"][n : n + 1, :].to_broadcast([P, M])
        )
        p_bc.append(pb)

    # ---------------- cosine content addressing ----------------
    # memT [W, M] via PE transposes of the 16 [128, W] memory blocks
    memT = const.tile([W, M], F32, tag="memT")
    for t in range(NT):
        tp = psA.tile([W, P], F32, tag="ps")
        nc.tensor.transpose(tp[:], mem[:, t * W : (t + 1) * W], ident[:])
        nc.scalar.copy(memT[:, t * P : (t + 1) * P], tp[:])

    # dot[h, m] = sum_w keys[h, w] * memory[m, w]
    dot_sb = small.tile([H, M], F32, tag="dot_sb")
    for c in range(2):  # two 1024-wide chunks, each = 2 matmuls of 512
        dp = psA.tile([H, M // 2], F32, tag="ps")
        for k in range(2):
            sl = slice(k * 512, (k + 1) * 512)
            gsl = slice(c * 1024 + k * 512, c * 1024 + (k + 1) * 512)
            nc.tensor.matmul(dp[:, sl], keysT[:], memT[:, gsl], start=True, stop=True)
        nc.scalar.copy(dot_sb[:, c * 1024 : (c + 1) * 1024], dp[:])
    # memT -> memT^2 (in place; scheduled after the dot matmuls read it)
    nc.scalar.activation(memT[:], memT[:], ACT.Square)
    # |mem|^2 broadcast over the H partitions, then sqrt -> sqnb
    sqnb = small.tile([H, M], F32, tag="sqnb")
    for c in range(2):
        nb = psA.tile([H, M // 2], F32, tag="ps")
        for k in range(2):
            sl = slice(k * 512, (k + 1) * 512)
            gsl = slice(c * 1024 + k * 512, c * 1024 + (k + 1) * 512)
            nc.tensor.matmul(nb[:, sl], ones_w_h[:], memT[:, gsl], start=True, stop=True)
        nc.scalar.activation(sqnb[:, c * 1024 : (c + 1) * 1024], nb[:], ACT.Sqrt)

    # key norms and softplus(strengths)
    keys2 = small.tile([H, W], F32, tag="keys2")
    kn2 = small.tile([H, 1], F32, tag="kn2")
    nc.scalar.activation(keys2[:], keys[:], ACT.Square, accum_out=kn2[:])
    kn = small.tile([H, 1], F32, tag="kn")
    nc.scalar.activation(kn[:], kn2[:], ACT.Sqrt)
    # softplus(x) = ln(1 + exp(x)); strengths ~ N(0,1) so exp won't overflow
    sp = small.tile([H, 1], F32, tag="sp")
    nc.scalar.activation(sp[:], stg[:], ACT.Exp)
    nc.scalar.activation(sp[:], sp[:], ACT.Ln, bias=1.0)

    # sqnb <- 1 / (|mem| * |key| + eps); dot_sb <- softmax(dot * sp * sqnb)
    nc.vector.tensor_scalar(sqnb[:], sqnb[:], kn[:], EPS, OP.mult, OP.add)
    nc.vector.reciprocal(sqnb[:], sqnb[:])
    nc.vector.scalar_tensor_tensor(dot_sb[:], dot_sb[:], sp[:], sqnb[:], OP.mult, OP.mult)
    negmx = small.tile([H, 1], F32, tag="negmx")
    nc.vector.tensor_reduce(negmx[:], dot_sb[:], AX.X, OP.max, negate=True)
    ssum = small.tile([H, 1], F32, tag="ssum")
    nc.scalar.activation(dot_sb[:], dot_sb[:], ACT.Exp, bias=negmx[:], accum_out=ssum[:])
    nc.vector.reciprocal(ssum[:], ssum[:])
    nc.vector.tensor_scalar(dot_sb[:], dot_sb[:], ssum[:], None, OP.mult)
    nc.sync.dma_start(outs["cosine_output"][:, :], dot_sb[:])

    # ---------------- precedence ----------------
    wsum = small.tile([NW, 1], F32, tag="wsum")
    nc.vector.tensor_reduce(wsum[:], ww[:], AX.X, OP.add)
    alpha = small.tile([NW, 1], F32, tag="alpha")
    nc.scalar.activation(alpha[:], wsum[:], ACT.Copy, bias=1.0, scale=-1.0)
    newp = small.tile([NW, M], F32, tag="newp")
    nc.vector.scalar_tensor_tensor(newp[:], pp[:], alpha[:], ww[:], OP.mult, OP.add)
    nc.sync.dma_start(outs["new_precedence"][:, :], newp[:])

    # ---------------- usage (partition products via ln/matmul/exp) ----------------
    lw = small.tile([NW, M], F32, tag="lw")
    nc.scalar.activation(lw[:], ww[:], ACT.Ln, bias=1.0, scale=-1.0)
    negfg = small.tile([NR, 1], F32, tag="negfg")
    nc.scalar.activation(negfg[:], fg[:], ACT.Copy, bias=0.0, scale=-1.0)
    lr = small.tile([NR, M], F32, tag="lr")
    nc.scalar.activation(lr[:], rw[:], ACT.Ln, bias=1.0, scale=negfg[:])
    ew = small.tile([1, M], F32, tag="ew")  # prod_n (1 - w_n)
    ret = small.tile([1, M], F32, tag="ret")  # retention
    for c in range(2):
        sps = psA.tile([1, M // 2], F32, tag="ps")
        rps = psA.tile([1, M // 2], F32, tag="ps")
        for k in range(2):
            sl = slice(k * 512, (k + 1) * 512)
            gsl = slice(c * 1024 + k * 512, c * 1024 + (k + 1) * 512)
            nc.tensor.matmul(sps[:, sl], ones_nw[:], lw[:, gsl], start=True, stop=True)
            nc.tensor.matmul(rps[:, sl], ones_nr[:], lr[:, gsl], start=True, stop=True)
        csl = slice(c * 1024, (c + 1) * 1024)
        nc.scalar.activation(ew[:, csl], sps[:], ACT.Exp)
        nc.scalar.activation(ret[:, csl], rps[:], ACT.Exp)
    # pu <- 1 - pu; ew <- 1 - ew*(1-pu); ew <- ew * ret  (= updated usage)
    nc.scalar.activation(pu[:], pu[:], ACT.Copy, bias=1.0, scale=-1.0)
    nc.vector.tensor_tensor(ew[:], ew[:], pu[:], OP.mult)
    nc.scalar.activation(ew[:], ew[:], ACT.Copy, bias=1.0, scale=-1.0)
    nc.vector.tensor_tensor(ew[:], ew[:], ret[:], OP.mult)
    nc.sync.dma_start(outs["updated_usage"][:, :], ew[:])

    # ---------------- erase / write ----------------
    er = small.tile([P, NT * W], F32, tag="er")
    eps_t = psW.tile([P, NT * W], F32, tag="psw")
    for t in range(NT):
        lhs = ww[:, t * P : (t + 1) * P]
        nc.tensor.matmul(eps_t[:, t * W : (t + 1) * W], lhs, ev[:], start=True, stop=True)
    nc.vector.tensor_scalar(er[:], eps_t[:], 0.0, 1.0, OP.max, OP.min)
    nc.scalar.activation(er[:], er[:], ACT.Copy, bias=1.0, scale=-1.0)  # 1 - erase
    nc.vector.tensor_tensor(er[:], mem[:], er[:], OP.mult)  # memory * (1 - erase)
    wps_t = psW.tile([P, NT * W], F32, tag="psw")
    for t in range(NT):
        lhs = ww[:, t * P : (t + 1) * P]
        nc.tensor.matmul(wps_t[:, t * W : (t + 1) * W], lhs, wv[:], start=True, stop=True)
    nc.vector.tensor_tensor(er[:], er[:], wps_t[:], OP.add)
    nc.sync.dma_start(
        outs["updated_memory"].rearrange("(t p) w -> p t w", p=P),
        er[:].rearrange("p (t w) -> p t w", t=NT),
    )

    # ---------------- read output ----------------
    rwsum = small.tile([NR, 1], F32, tag="rwsum")
    nc.vector.tensor_reduce(rwsum[:], rw[:], AX.X, OP.add)
    wv4 = small.tile([NR, W], F32, tag="wv4")
    nc.vector.tensor_scalar(wv4[:], rv[:], rwsum[:], None, OP.mult)
    rops = psA.tile([1, W], F32, tag="ps")
    nc.tensor.matmul(rops[:], ones_nr[:], wv4[:], start=True, stop=True)
    ro = small.tile([1, W], F32, tag="ro")
    nc.scalar.copy(ro[:], rops[:])
    nc.sync.dma_start(outs["read_output"][:, :], ro[:])

    # ---------------- link matrix stream (the 64MB/core long pole) ----------------
    for n in range(NW):
        for t in range(NT):
            rows = slice(t * P, (t + 1) * P)
            lin = lpool.tile([P, M], F32, tag="lin")
            nc.sync.dma_start(lin[:], ins["prev_link"][n, rows, :])
            lo = opool.tile([P, M], F32, tag="lo")
            wcol = w_col[:, n * NT + t : n * NT + t + 1]
            # t0 = ((1 - w_j) - w_i) * L
            nc.vector.scalar_tensor_tensor(
                lo[:], a_bc[n][:], wcol, lin[:], OP.subtract, OP.mult
            )
            # out = (p_j * w_i) + t0
            nc.vector.scalar_tensor_tensor(
                lo[:], p_bc[n][:], wcol, lo[:], OP.mult, OP.add
            )
            # zero the diagonal block
            dsl = slice(t * P, (t + 1) * P)
            nc.vector.tensor_tensor(lo[:, dsl], lo[:, dsl], dmask[:], OP.mult)
            nc.sync.dma_start(outs["new_link"][n, rows, :], lo[:])


_CACHE: dict = {}


def build_nc():
    if "nc" in _CACHE:
        return _CACHE["nc"]
    nc = bacc.Bacc("TRN2", target_bir_lowering=False, debug=False, num_devices=B)
    ishapes = {
        "memory": [M, W],
        "keys": [H, W],
        "strengths": [H, 1],
        "write_weights": [NW, M],
        "free_gate": [NR, 1],
        "read_weights": [NR, M],
        "prev_link": [NW, M, M],
        "prev_precedence": [NW, M],
        "prev_usage": [1, M],
        "erase_vectors": [NW, W],
        "write_vectors": [NW, W],
        "read_vectors": [NR, W],
    }
    oshapes = {
        "updated_memory": [M, W],
        "cosine_output": [H, M],
        "new_link": [NW, M, M],
        "new_precedence": [NW, M],
        "updated_usage": [1, M],
        "read_output": [1, W],
    }
    ins = {
        k: nc.dram_tensor(k, v, F32, kind="ExternalInput").ap()
        for k, v in ishapes.items()
    }
    outs = {
        k: nc.dram_tensor(k, v, F32, kind="ExternalOutput").ap()
        for k, v in oshapes.items()
    }
    with tile.TileContext(nc) as tc:
        with ExitStack() as ctx:
            _emit(ctx, tc, ins, outs)
    nc.compile()
    _CACHE["nc"] = nc
    return nc


def make_in_maps(inputs: dict) -> list[dict]:
    in_maps = []
    for b in range(B):
        in_maps.append(
            {
                "memory": np.ascontiguousarray(inputs["memory"][b]),
                "keys": np.ascontiguousarray(inputs["keys"][b]),
                "strengths": np.ascontiguousarray(
                    inputs["strengths"][b].reshape(H, 1)
                ),
                "write_weights": np.ascontiguousarray(inputs["write_weights"][b]),
                "free_gate": np.ascontiguousarray(
                    inputs["free_gate"][b].reshape(NR, 1)
                ),
                "read_weights": np.ascontiguousarray(inputs["read_weights"][b]),
                "prev_link": np.ascontiguousarray(inputs["prev_link"][b]),
                "prev_precedence": np.ascontiguousarray(
                    inputs["prev_precedence"][b]
                ),
                "prev_usage": np.ascontiguousarray(
                    inputs["prev_usage"][b].reshape(1, M)
                ),
                "erase_vectors": np.ascontiguousarray(inputs["erase_vectors"][b]),
                "write_vectors": np.ascontiguousarray(inputs["write_vectors"][b]),
                "read_vectors": np.ascontiguousarray(inputs["read_vectors"][b]),
            }
        )
    return in_maps


def assemble(results: list[dict]) -> tuple:
    upd_mem = np.stack([results[b]["updated_memory"] for b in range(B)])
    cos = np.stack([results[b]["cosine_output"] for b in range(B)])
    link = np.stack([results[b]["new_link"] for b in range(B)])
    prec = np.stack([results[b]["new_precedence"] for b in range(B)])
    usage = np.stack([results[b]["updated_usage"].reshape(M) for b in range(B)])
    ro = np.stack([results[b]["read_output"].reshape(W) for b in range(B)])
    return upd_mem, cos, link, prec, usage, ro


def kernel(**inputs):
    nc = build_nc()
    res = run_bass_kernel_spmd(nc, make_in_maps(inputs), list(range(B))).results
    return assemble(res)
